# revision 1
# baseline (speedup 1.0000x reference)
"""Data-parallel GeneratedTreeClassifier forward on 8 NeuronCores (Bass/Tile).

Shards the batch dim of x (16384 -> 8 x 2048) across cores, replicates the
small tree params, runs a hand-written Bass/Tile kernel per core, and
gathers the full [16384, 512] output.

Math restructure (per tree t, decision i = 0..3, r = 1/(4 + d3 + eps)):
  out = leaf_norm @ (softmax(leaf_logits) * w)
      = r @ C + (r*d) @ G
  C_t = M_t0 + M_t2 + M_t4 + M_t6
  G_(t,i) = M_t(1+2i) - M_t(2+2i)  (i<3),   G_(t,3) = M_t7
which kills the per-tile leaf assembly + transpose and shrinks mm2's K
from 512 to 320.

Per-core device graph, processed in groups of 4 batch tiles (512 rows):
  xT   <- PE transpose (identity matmul) of bf16 x tiles  [128d, 4k, 512b]
  zT   = W @ x^T            (PE, j-major: 8 matmuls of N=512)
  d    = sigmoid(zT + bias) (ACT, bias per-partition)     [128j, 2, 512b]
  r    = 1/(4+d3+eps)       (DVE approx, partitions 64:128)
  e    = d * r              (DVE, bf16)   -> mm2 lhsT tiles T0, T1
  out  = [e; r] @ [G; C]    (PE, 3 matmuls of N=512 per batch tile)
  M    = softmax(leaf_logits)*w, C/G via pattern matmuls (one-time).
"""
import numpy as np
import ml_dtypes
from contextlib import ExitStack

import concourse.bass as bass
import concourse.tile as tile
from concourse import bacc, mybir

INPUT_DIM = 512
N_CLASSES = 512
N_TREES = 64
N_LEAVES = 8
N_INTERNAL = 7
PPT = N_INTERNAL * (INPUT_DIM + 1) + N_LEAVES * N_CLASSES
BATCH = 16384
N_CORES = 8
BSH = BATCH // N_CORES          # 2048 rows per core
NB = BSH // 128                 # 16 batch tiles per core
NG = NB // 4                    # 4 groups of 4 tiles
NW = N_INTERNAL * INPUT_DIM
EPS = 1e-8

F32 = mybir.dt.float32
BF16 = mybir.dt.bfloat16


def _emit(ctx: ExitStack, tc, xt, wT, bb, bb2, ll, wr, pc, pg, out):
    nc = tc.nc
    AF = mybir.ActivationFunctionType

    const = ctx.enter_context(tc.tile_pool(name="const", bufs=1))

    # Replicated params, resident in SBUF.
    wt_sb = const.tile([128, 4, 256], BF16)          # [d%128, dk, j]
    nc.sync.dma_start(wt_sb[:], wT.rearrange("(k p) j -> p k j", p=128))
    bias_sb = const.tile([128, 2], F32)              # [j%128, jb]
    nc.sync.dma_start(bias_sb[:], bb.rearrange("(jb p) one -> p (jb one)", p=128))
    bias2_sb = const.tile([128, 1], F32)             # b3 + ln(1.25) at 64:128
    nc.sync.dma_start(bias2_sb[64:128, :], bb2[:])
    pc_sb = const.tile([128, 4, 64], BF16)           # [tl%128, tlk, t]
    nc.sync.dma_start(pc_sb[:], pc.rearrange("(k p) t -> p k t", p=128))
    pg_sb = const.tile([128, 4, 256], BF16)          # [tl%128, tlk, j]
    nc.sync.dma_start(pg_sb[:], pg.rearrange("(k p) j -> p k j", p=128))
    m_sb = const.tile([128, 4, N_CLASSES], BF16)     # [tl%128, tlk, c]
    cg0 = const.tile([128, N_CLASSES], BF16)         # G rows (i0; i1)
    cg1 = const.tile([128, N_CLASSES], BF16)         # G rows (i2; i3)
    cg2 = const.tile([128, N_CLASSES], BF16)         # C rows at 64:128

    # M = softmax(leaf_logits, axis=-1) * w_tree   (rows tl = t*8 + l)
    ppool = ctx.enter_context(tc.tile_pool(name="prm", bufs=2))
    for k in range(4):
        llt = ppool.tile([128, N_CLASSES], F32, tag="llt")
        nc.sync.dma_start(llt[:], ll[k * 128:(k + 1) * 128, :])
        e = ppool.tile([128, N_CLASSES], F32, tag="e")
        s = ppool.tile([128, 1], F32, tag="s")
        nc.scalar.activation(e[:], llt[:], AF.Exp,
                             bias=0.0, scale=1.0, accum_out=s[:])
        rs = ppool.tile([128, 1], F32, tag="rs")
        nc.vector.reciprocal(rs[:], s[:])
        wrt = ppool.tile([128, 1], F32, tag="wrt")
        nc.sync.dma_start(wrt[:], wr[k * 128:(k + 1) * 128, :])
        sc = ppool.tile([128, 1], F32, tag="sc")
        nc.vector.tensor_tensor(sc[:], rs[:], wrt[:], op=mybir.AluOpType.mult)
        nc.vector.tensor_scalar_mul(m_sb[:, k, :], e[:], sc[:])

    spool = ctx.enter_context(tc.tile_pool(name="xT", bufs=1))
    dpool = ctx.enter_context(tc.tile_pool(name="work", bufs=3))
    epool = ctx.enter_context(tc.tile_pool(name="eT", bufs=3))
    opool = ctx.enter_context(tc.tile_pool(name="osb", bufs=6))
    zpp = ctx.enter_context(tc.tile_pool(name="zps", bufs=4, space="PSUM"))
    opp = ctx.enter_context(tc.tile_pool(name="ops", bufs=4, space="PSUM"))

    # One-time: C/G from M via host-provided 0/±1 pattern matrices.
    cg2ps = opp.tile([128, 512], F32, tag="o")
    for k in range(4):
        nc.tensor.matmul(cg2ps[64:128, :], lhsT=pc_sb[:, k, :],
                         rhs=m_sb[:, k, :], start=(k == 0), stop=(k == 3))
    nc.scalar.copy(cg2[64:128, :], cg2ps[64:128, :])
    cg0ps = opp.tile([128, 512], F32, tag="o")
    for k in range(4):
        nc.tensor.matmul(cg0ps[:], lhsT=pg_sb[:, k, 0:128],
                         rhs=m_sb[:, k, :], start=(k == 0), stop=(k == 3))
    nc.scalar.copy(cg0[:], cg0ps[:])
    cg1ps = opp.tile([128, 512], F32, tag="o")
    for k in range(4):
        nc.tensor.matmul(cg1ps[:], lhsT=pg_sb[:, k, 128:256],
                         rhs=m_sb[:, k, :], start=(k == 0), stop=(k == 3))
    nc.scalar.copy(cg1[:], cg1ps[:])

    # x^T resident in SBUF (pre-transposed on host): xT[p, k, b] = x[b, k*128+p]
    xT = spool.tile([128, 4, BSH], BF16)
    for k, eng in enumerate((nc.sync, nc.scalar, nc.gpsimd, nc.sync)):
        eng.dma_start(xT[:, k, :], xt[k * 128:(k + 1) * 128, :])

    for g in range(NG):
        # zT[j, b] = sum_d W[j, d] x[b, d]    j = i*64 + t, i-major
        gs = slice(g * 512, (g + 1) * 512)
        zt0 = zpp.tile([128, 512], F32, tag="zt")
        zt1 = zpp.tile([128, 512], F32, tag="zt")
        for jb, ztile in enumerate((zt0, zt1)):
            for idx, k in enumerate((1, 2, 0, 3)):
                nc.tensor.matmul(ztile[:],
                                 lhsT=wt_sb[:, k, jb * 128:(jb + 1) * 128],
                                 rhs=xT[:, k, gs],
                                 start=(idx == 0), stop=(idx == 3))
        d4 = dpool.tile([128, 2, 512], BF16, tag="d4")
        for jb, ztile in enumerate((zt0, zt1)):
            nc.scalar.activation(d4[:, jb, :], ztile[:], AF.Sigmoid,
                                 bias=bias_sb[:, jb:jb + 1])

        # r = 1/(4 + d3) = 1/4 - sigmoid(z3 + ln 1.25)/20   (exact identity)
        s3 = dpool.tile([128, 512], F32, tag="s3")
        nc.scalar.activation(s3[64:128, :], zt1[64:128, :], AF.Sigmoid,
                             bias=bias2_sb[64:128, :])
        rb = dpool.tile([128, 512], BF16, tag="rb")
        nc.vector.tensor_scalar(rb[64:128, :], s3[64:128, :], -0.05, 0.25,
                                op0=mybir.AluOpType.mult,
                                op1=mybir.AluOpType.add)
        # replicate r to all (i, jb) lanes:  r4[a*64+t, jb, b] = r[t, b]
        r4 = dpool.tile([128, 2, 512], BF16, tag="r4")
        for jb in range(2):
            for a in range(2):
                nc.gpsimd.dma_start(r4[a * 64:(a + 1) * 64, jb, :],
                                    rb[64:128, :])

        # e = d * r  -> lhsT tiles for mm2 (rows i*64+t match G rows)
        T0 = epool.tile([128, 512], BF16, tag="T0")
        T1 = epool.tile([128, 512], BF16, tag="T1")
        nc.vector.tensor_tensor(T0[:], d4[:, 0, :], r4[:, 0, :],
                                op=mybir.AluOpType.mult)
        nc.vector.tensor_tensor(T1[:], d4[:, 1, :], r4[:, 1, :],
                                op=mybir.AluOpType.mult)

        # out = e @ G + r @ C  per batch tile
        for bt in range(4):
            bs = slice(bt * 128, (bt + 1) * 128)
            ops = opp.tile([128, 512], F32, tag="o")
            nc.tensor.matmul(ops[:], lhsT=T0[:, bs], rhs=cg0[:],
                             start=True, stop=False)
            nc.tensor.matmul(ops[:], lhsT=T1[:, bs], rhs=cg1[:],
                             start=False, stop=False)
            nc.tensor.matmul(ops[:], lhsT=rb[64:128, bs], rhs=cg2[64:128, :],
                             start=False, stop=True)
            osb = opool.tile([128, 512], BF16, tag="osb")
            if bt % 2 == 0:
                nc.scalar.copy(osb[:], ops[:])
            else:
                nc.vector.tensor_copy(osb[:], ops[:])
            deng = nc.sync if bt % 2 == 0 else nc.gpsimd
            deng.dma_start(out[(4 * g + bt) * 128:(4 * g + bt + 1) * 128, :],
                           osb[:])


_NC = None
_RUNNER = None


def _get_nc():
    global _NC
    if _NC is None:
        nc = bacc.Bacc("TRN2", target_bir_lowering=False, debug=False)
        xt = nc.dram_tensor("xt", [INPUT_DIM, BSH], BF16, kind="ExternalInput")
        wT = nc.dram_tensor("wT", [INPUT_DIM, 256], BF16, kind="ExternalInput")
        bb = nc.dram_tensor("bb", [256, 1], F32, kind="ExternalInput")
        bb2 = nc.dram_tensor("bb2", [64, 1], F32, kind="ExternalInput")
        ll = nc.dram_tensor("ll", [512, N_CLASSES], F32, kind="ExternalInput")
        wr = nc.dram_tensor("wr", [512, 1], F32, kind="ExternalInput")
        pc = nc.dram_tensor("pc", [512, 64], BF16, kind="ExternalInput")
        pg = nc.dram_tensor("pg", [512, 256], BF16, kind="ExternalInput")
        out = nc.dram_tensor("out", [BSH, N_CLASSES], BF16, kind="ExternalOutput")
        with tile.TileContext(nc) as tc, ExitStack() as ctx:
            _emit(ctx, tc, xt.ap(), wT.ap(), bb.ap(), bb2.ap(), ll.ap(), wr.ap(),
                  pc.ap(), pg.ap(), out.ap())
        nc.compile()
        _NC = nc
    return _NC


def _get_runner():
    """Build the sharded PJRT executable ONCE (jit + NEFF compile are cached
    across kernel() calls; run_bass_kernel_spmd would re-trace every call)."""
    global _RUNNER
    if _RUNNER is None:
        import jax
        import jax.numpy as jnp
        from jax.sharding import Mesh, PartitionSpec, NamedSharding
        from jax.experimental.shard_map import shard_map
        from concourse import bass2jax

        nc = _get_nc()
        bass2jax.install_neuronx_cc_hook()

        part_name = (nc.partition_id_tensor.name
                     if nc.partition_id_tensor else None)
        in_names, out_names, out_avals = [], [], []
        for alloc in nc.m.functions[0].allocations:
            if not isinstance(alloc, mybir.MemoryLocationSet):
                continue
            name = alloc.memorylocations[0].name
            if alloc.kind == "ExternalInput":
                if name != part_name:
                    in_names.append(name)
            elif alloc.kind == "ExternalOutput":
                out_names.append(name)
                out_avals.append(jax.core.ShapedArray(
                    tuple(alloc.tensor_shape), mybir.dt.np(alloc.dtype)))
        n_params = len(in_names)
        all_names = tuple(in_names) + tuple(out_names)
        if part_name is not None:
            all_names = all_names + (part_name,)
        donate = tuple(range(n_params, n_params + len(out_names)))

        def _body(*args):
            operands = list(args)
            if part_name is not None:
                operands.append(bass2jax.partition_id_tensor())
            outs = bass2jax._bass_exec_p.bind(
                *operands,
                out_avals=tuple(out_avals),
                in_names=all_names,
                out_names=tuple(out_names),
                lowering_input_output_aliases=(),
                sim_require_finite=True,
                sim_require_nnan=True,
                nc=nc,
            )
            return tuple(outs)

        devices = jax.devices()[:N_CORES]
        mesh = Mesh(np.asarray(devices), ("core",))
        spec = PartitionSpec("core")
        fn = jax.jit(
            shard_map(_body, mesh=mesh,
                      in_specs=(spec,) * (n_params + len(out_names)),
                      out_specs=(spec,) * len(out_names), check_rep=False),
            donate_argnums=donate, keep_unused=True)
        zmk = jax.jit(
            lambda: jnp.zeros((N_CORES * BSH, N_CLASSES), ml_dtypes.bfloat16),
            out_shardings=NamedSharding(mesh, spec))
        _RUNNER = (fn, zmk, in_names)
    return _RUNNER


def _patterns():
    """0/±1 combination matrices: C = PC^T M, G = PG^T M (tl = 8t + l)."""
    pcm = np.zeros((512, 64), np.float32)
    pgm = np.zeros((512, 256), np.float32)
    for t in range(N_TREES):
        for l in (0, 2, 4, 6):
            pcm[8 * t + l, t] = 1.0
        for i in range(3):
            pgm[8 * t + 1 + 2 * i, i * 64 + t] = 1.0
            pgm[8 * t + 2 + 2 * i, i * 64 + t] = -1.0
        pgm[8 * t + 7, 3 * 64 + t] = 1.0
    return (pcm.astype(ml_dtypes.bfloat16), pgm.astype(ml_dtypes.bfloat16))


_PC, _PG = _patterns()


def _host_prep(x, tree_params, tree_weights):
    """Slice/layout the replicated params and cast x to bf16 (host-side)."""
    x = np.asarray(x, np.float32).astype(ml_dtypes.bfloat16)
    xt = np.empty((N_CORES * INPUT_DIM, BSH), ml_dtypes.bfloat16)
    for c in range(N_CORES):
        xt[c * INPUT_DIM:(c + 1) * INPUT_DIM] = x[c * BSH:(c + 1) * BSH].T
    p = np.asarray(tree_params, np.float32)[0].reshape(N_TREES, PPT)
    w = p[:, :NW].reshape(N_TREES, N_INTERNAL, INPUT_DIM)[:, :4, :]
    # j = i*64 + t (i-major)
    w_im = np.ascontiguousarray(w.transpose(1, 0, 2).reshape(256, INPUT_DIM))
    wT = np.ascontiguousarray(w_im.T).astype(ml_dtypes.bfloat16)
    bias = np.ascontiguousarray(
        p[:, NW:NW + N_INTERNAL][:, :4].T.reshape(256, 1))
    bias2 = np.ascontiguousarray(bias[192:256] + np.float32(np.log(1.25)))
    ll = np.ascontiguousarray(p[:, NW + N_INTERNAL:].reshape(512, N_CLASSES))
    wr = np.repeat(np.asarray(tree_weights, np.float32)[0], N_LEAVES)
    wr = np.ascontiguousarray(wr.reshape(512, 1))
    return xt, wT, bias, bias2, ll, wr


def kernel(x: np.ndarray, tree_params: np.ndarray,
           tree_weights: np.ndarray) -> np.ndarray:
    fn, zmk, in_names = _get_runner()
    xbf, wT, bias, bias2, ll, wr = _host_prep(x, tree_params, tree_weights)
    reps = {"xt": xbf,
            "wT": np.concatenate([wT] * N_CORES, 0),
            "bb": np.concatenate([bias] * N_CORES, 0),
            "bb2": np.concatenate([bias2] * N_CORES, 0),
            "ll": np.concatenate([ll] * N_CORES, 0),
            "wr": np.concatenate([wr] * N_CORES, 0),
            "pc": np.concatenate([_PC] * N_CORES, 0),
            "pg": np.concatenate([_PG] * N_CORES, 0)}
    args = [reps[n] for n in in_names] + [zmk()]
    outs = fn(*args)
    return np.asarray(outs[0]).astype(np.float32)



# revision 2
# speedup vs baseline: 1.3211x; 1.3211x over previous
"""Data-parallel GeneratedTreeClassifier forward on 8 NeuronCores (Bass/Tile).

Shards the batch dim of x (16384 -> 8 x 2048) across cores, replicates the
small tree params, runs a hand-written Bass/Tile kernel per core, and
gathers the full [16384, 512] output.

Math restructure (per tree t, decisions i = 0..3, r = 1/(4 + d3)):
  leaf_probs = r * [1, d0, 1-d0, d1, 1-d1, d2, 1-d2, d3]
  out = sum_t  r C_t + sum_{i<3} (r d_i) G_ti + (r d3) M_t7
  with  e_i = d_i r  and the exact identity  e3 = 1 - 4 r:
  out = S[c] + e @ [G0; G1; G2; G3']      (K = 256)
  G3' = M_t7 - C_t/4,  S[c] = sum_t C_t[c]/4   (rank-1, added on HOST)
  where M = softmax(leaf_logits) * w_tree, C/G row-combines of M are
  all precomputed on the host (cg0 = [G0;G1], cg1 = [G2;G3']).

Per-core device graph (4 groups of 512 batch rows):
  zT   = W @ x^T                (PE, 8 matmuls N=512; x^T staged on host)
  d    = sigmoid(zT + bias)     (ACT, bias per-partition)
  r    = 1/4 - sigmoid(z3 + ln1.25)/20   (exact identity, ACT+DVE)
  rf   = [r; r]                 (one 64-partition SBUF->SBUF DMA)
  T0   = d01 * rf, T1 = d23 * rf  (DVE, bf16)
  out  = T0 @ cg0 + T1 @ cg1    (PE, 2 matmuls N=512 per batch tile)
"""
import numpy as np
import ml_dtypes
from contextlib import ExitStack

import concourse.bass as bass
import concourse.tile as tile
from concourse import bacc, mybir

INPUT_DIM = 512
N_CLASSES = 512
N_TREES = 64
N_LEAVES = 8
N_INTERNAL = 7
PPT = N_INTERNAL * (INPUT_DIM + 1) + N_LEAVES * N_CLASSES
BATCH = 16384
N_CORES = 8
BSH = BATCH // N_CORES          # 2048 rows per core
NG = 4                          # 4 groups of 512 rows
NW = N_INTERNAL * INPUT_DIM

F32 = mybir.dt.float32
BF16 = mybir.dt.bfloat16


def _emit(ctx: ExitStack, tc, xt, pbf, pf32, out):
    nc = tc.nc
    AF = mybir.ActivationFunctionType
    MUL = mybir.AluOpType.mult
    ADD = mybir.AluOpType.add

    const = ctx.enter_context(tc.tile_pool(name="const", bufs=1))

    # Packed bf16 params, resident in SBUF:
    #   [:, k*256 + jb*128 + j'] = W^T chunk (d = k*128+p, j = jb*128+j')
    #   [:, 1024:1536] = cg0 rows i*64+t (i=0,1), [:, 1536:2048] = cg1 (i=2,3')
    pb = const.tile([128, 2048], BF16)
    nc.sync.dma_start(pb[:], pbf[:])
    cg0 = pb[:, 1024:1536]
    cg1 = pb[:, 1536:2048]
    # f32 params: col 0/1 = sigmoid bias (j-blocks), col 2 = b3 + ln(1.25)
    pf = const.tile([128, 3], F32)
    nc.sync.dma_start(pf[:], pf32[:])

    # x^T resident in SBUF (pre-transposed per group on host):
    #   xT[p, 4g+k, b] = x[g*512 + b, k*128 + p]
    xT = const.tile([128, 4 * NG, 512], BF16)
    for g, eng in enumerate((nc.sync, nc.gpsimd, nc.gpsimd, nc.sync)):
        eng.dma_start(xT[:, 4 * g:4 * g + 4, :],
                      xt[g * 512:(g + 1) * 512, :].rearrange(
                          "(k p) b -> p k b", p=128))

    dpool = ctx.enter_context(tc.tile_pool(name="work", bufs=2))
    epool = ctx.enter_context(tc.tile_pool(name="eT", bufs=2))
    opool = ctx.enter_context(tc.tile_pool(name="osb", bufs=2))
    zpp = ctx.enter_context(tc.tile_pool(name="zps", bufs=2, space="PSUM"))
    opp = ctx.enter_context(tc.tile_pool(name="ops", bufs=4, space="PSUM"))

    for g in range(NG):
        # zT[j, b] = sum_d W[j, d] x[b, d]    j = i*64 + t, i-major
        zt0 = zpp.tile([128, 512], F32, tag="zt0")
        zt1 = zpp.tile([128, 512], F32, tag="zt1")
        for jb, zt in ((0, zt0), (1, zt1)):
            for k in range(4):
                nc.tensor.matmul(zt[:],
                                 lhsT=pb[:, k * 256 + jb * 128:
                                         k * 256 + (jb + 1) * 128],
                                 rhs=xT[:, 4 * g + k, :],
                                 start=(k == 0), stop=(k == 3))
        d0 = dpool.tile([128, 512], BF16, tag="d0")
        d1 = dpool.tile([128, 512], BF16, tag="d1")
        nc.scalar.activation(d0[:], zt0[:], AF.Sigmoid, bias=pf[:, 0:1])
        nc.scalar.activation(d1[:], zt1[:], AF.Sigmoid, bias=pf[:, 1:2])

        # r = 1/(4 + d3) = 1/4 - sigmoid(z3 + ln 1.25)/20   (exact identity)
        s3 = dpool.tile([128, 512], F32, tag="s3")
        nc.scalar.activation(s3[64:128, :], zt1[64:128, :], AF.Sigmoid,
                             bias=pf[64:128, 2:3])
        rf = dpool.tile([128, 512], BF16, tag="rf")
        nc.vector.tensor_scalar(rf[64:128, :], s3[64:128, :], -0.05, 0.25,
                                op0=MUL, op1=ADD)
        # replicate r to the low 64 partitions (one small SBUF->SBUF DMA)
        nc.gpsimd.dma_start(rf[0:64, :], rf[64:128, :])

        # e = d * r  -> lhsT tiles for mm2 (rows i*64+t match cg rows)
        T0 = epool.tile([128, 512], BF16, tag="T0")
        T1 = epool.tile([128, 512], BF16, tag="T1")
        nc.vector.tensor_tensor(T0[:], d0[:], rf[:], op=MUL)
        nc.vector.tensor_tensor(T1[:], d1[:], rf[:], op=MUL)

        # out = T0 @ cg0 + T1 @ cg1  per batch tile
        osb = opool.tile([128, 4, 512], BF16, tag="osb")
        for bt in range(4):
            bs = slice(bt * 128, (bt + 1) * 128)
            ops = opp.tile([128, 512], F32, tag="o")
            nc.tensor.matmul(ops[:], lhsT=T0[:, bs], rhs=cg0,
                             start=True, stop=False)
            nc.tensor.matmul(ops[:], lhsT=T1[:, bs], rhs=cg1,
                             start=False, stop=True)
            nc.vector.tensor_copy(osb[:, bt, :], ops[:])
        deng = nc.sync if g % 2 == 0 else nc.gpsimd
        deng.dma_start(out[g * 512:(g + 1) * 512, :].rearrange(
            "(bt p) c -> p bt c", p=128), osb[:])


_NC = None
_RUNNER = None


def _get_nc():
    global _NC
    if _NC is None:
        nc = bacc.Bacc("TRN2", target_bir_lowering=False, debug=False)
        xt = nc.dram_tensor("xt", [BSH, 512], BF16, kind="ExternalInput")
        pbf = nc.dram_tensor("pbf", [128, 2048], BF16, kind="ExternalInput")
        pf32 = nc.dram_tensor("pf32", [128, 3], F32, kind="ExternalInput")
        out = nc.dram_tensor("out", [BSH, N_CLASSES], BF16, kind="ExternalOutput")
        with tile.TileContext(nc) as tc, ExitStack() as ctx:
            _emit(ctx, tc, xt.ap(), pbf.ap(), pf32.ap(), out.ap())
        nc.compile()
        _NC = nc
    return _NC


def _get_runner():
    """Build the sharded PJRT executable ONCE (jit + NEFF compile are cached
    across kernel() calls; run_bass_kernel_spmd would re-trace every call)."""
    global _RUNNER
    if _RUNNER is None:
        import jax
        import jax.numpy as jnp
        from jax.sharding import Mesh, PartitionSpec, NamedSharding
        from jax.experimental.shard_map import shard_map
        from concourse import bass2jax

        nc = _get_nc()
        bass2jax.install_neuronx_cc_hook()

        part_name = (nc.partition_id_tensor.name
                     if nc.partition_id_tensor else None)
        in_names, out_names, out_avals = [], [], []
        for alloc in nc.m.functions[0].allocations:
            if not isinstance(alloc, mybir.MemoryLocationSet):
                continue
            name = alloc.memorylocations[0].name
            if alloc.kind == "ExternalInput":
                if name != part_name:
                    in_names.append(name)
            elif alloc.kind == "ExternalOutput":
                out_names.append(name)
                out_avals.append(jax.core.ShapedArray(
                    tuple(alloc.tensor_shape), mybir.dt.np(alloc.dtype)))
        n_params = len(in_names)
        all_names = tuple(in_names) + tuple(out_names)
        if part_name is not None:
            all_names = all_names + (part_name,)
        donate = tuple(range(n_params, n_params + len(out_names)))

        def _body(*args):
            operands = list(args)
            if part_name is not None:
                operands.append(bass2jax.partition_id_tensor())
            outs = bass2jax._bass_exec_p.bind(
                *operands,
                out_avals=tuple(out_avals),
                in_names=all_names,
                out_names=tuple(out_names),
                lowering_input_output_aliases=(),
                sim_require_finite=True,
                sim_require_nnan=True,
                nc=nc,
            )
            return tuple(outs)

        devices = jax.devices()[:N_CORES]
        mesh = Mesh(np.asarray(devices), ("core",))
        spec = PartitionSpec("core")
        fn = jax.jit(
            shard_map(_body, mesh=mesh,
                      in_specs=(spec,) * (n_params + len(out_names)),
                      out_specs=(spec,) * len(out_names), check_rep=False),
            donate_argnums=donate, keep_unused=True)
        zmk = jax.jit(
            lambda: jnp.zeros((N_CORES * BSH, N_CLASSES), ml_dtypes.bfloat16),
            out_shardings=NamedSharding(mesh, spec))
        _RUNNER = (fn, zmk, in_names)
    return _RUNNER


def _host_prep(x, tree_params, tree_weights):
    """Host-side: transpose/group x, pack replicated params, and fold the
    leaf-distribution combination matrices (incl. softmax) plus the rank-1
    output shift S into precomputed arrays."""
    x = np.asarray(x, np.float32).astype(ml_dtypes.bfloat16)
    # xt[(g k p), b] = x_core[g*512 + b, k*128 + p], per core
    xt = np.ascontiguousarray(
        x.reshape(N_CORES, NG, 512, 4, 128).transpose(0, 1, 3, 4, 2)
    ).reshape(N_CORES * BSH, 512)

    p = np.asarray(tree_params, np.float32)[0].reshape(N_TREES, PPT)
    w4 = p[:, :NW].reshape(N_TREES, N_INTERNAL, INPUT_DIM)[:, :4, :]
    wj = w4.transpose(1, 0, 2).reshape(256, INPUT_DIM)      # j = i*64 + t
    # wt packed [128, 4k, 256j] -> [128, 1024]
    wt = np.ascontiguousarray(
        wj.T.reshape(4, 128, 256).transpose(1, 0, 2)).reshape(128, 1024)

    ll = p[:, NW + N_INTERNAL:].reshape(N_TREES, N_LEAVES, N_CLASSES)
    e = np.exp(ll - ll.max(axis=-1, keepdims=True))
    M = e / e.sum(axis=-1, keepdims=True)                   # softmax [T, L, C]
    M = M * np.asarray(tree_weights, np.float32)[0][:, None, None]
    C_ = M[:, 0] + M[:, 2] + M[:, 4] + M[:, 6]              # [T, C]
    G0 = M[:, 1] - M[:, 2]
    G1 = M[:, 3] - M[:, 4]
    G2 = M[:, 5] - M[:, 6]
    G3 = M[:, 7] - C_ * 0.25
    cg0 = np.concatenate([G0, G1], 0)                       # [128, C]
    cg1 = np.concatenate([G2, G3], 0)
    pbf = np.concatenate(
        [wt, cg0.astype(np.float32), cg1.astype(np.float32)],
        axis=1).astype(ml_dtypes.bfloat16)                  # [128, 2048]

    bias = p[:, NW:NW + N_INTERNAL][:, :4].T.reshape(256)   # j-major
    pf32 = np.zeros((128, 3), np.float32)
    pf32[:, 0] = bias[0:128]
    pf32[:, 1] = bias[128:256]
    pf32[64:128, 2] = bias[192:256] + np.float32(np.log(1.25))

    S = C_.sum(axis=0) * 0.25                               # [C] host shift
    return xt, pbf, pf32, S


def kernel(x: np.ndarray, tree_params: np.ndarray,
           tree_weights: np.ndarray) -> np.ndarray:
    fn, zmk, in_names = _get_runner()
    xt, pbf, pf32, S = _host_prep(x, tree_params, tree_weights)
    reps = {"xt": xt,
            "pbf": np.concatenate([pbf] * N_CORES, 0),
            "pf32": np.concatenate([pf32] * N_CORES, 0)}
    args = [reps[n] for n in in_names] + [zmk()]
    outs = fn(*args)
    return np.asarray(outs[0]).astype(np.float32) + S[None, :]


# revision 3
# speedup vs baseline: 1.6063x; 1.2159x over previous
"""Data-parallel GeneratedTreeClassifier forward on 8 NeuronCores (Bass/Tile).

Shards the batch dim of x (16384 -> 8 x 2048) across cores, replicates the
small tree params, runs a hand-written Bass/Tile kernel per core, and
gathers the full [16384, 512] output.

Math restructure (per tree t, decisions i = 0..3, r = 1/(4 + d3)):
  leaf_probs = r * [1, d0, 1-d0, d1, 1-d1, d2, 1-d2, d3]
  out = sum_t  r C_t + sum_{i<3} (r d_i) G_ti + (r d3) M_t7
  with  e_i = d_i r  and the exact identity  e3 = 1 - 4 r:
  out = S[c] + e @ [G0; G1; G2; G3']      (K = 256)
  G3' = M_t7 - C_t/4,  S[c] = sum_t C_t[c]/4   (rank-1, added on HOST)
  where M = softmax(leaf_logits) * w_tree; C/G row-combines of M are all
  precomputed on the host (cg0 = [G0;G1], cg1 = [G2;G3']).

Per-core device graph (4 groups of 512 batch rows, software-pipelined so
mm1 of group g+1 runs on the PE while group g's ACT/DVE chain computes):
  zT   = W @ x^T                 (PE, 8 matmuls N=512; x^T staged on host)
  d    = sigmoid(zT + bias)      (ACT, bias per-partition)
  r    = 1/4 - sigmoid(z3 + ln1.25)/20    (exact identity; the two rf
         halves are written by partition-shifted DVE tensor_scalar ops,
         no broadcast DMA)
  T0   = d01 * rf, T1 = d23 * rf (DVE, bf16)
  out  = T0 @ cg0 + T1 @ cg1     (PE, 2 matmuls N=512 per batch tile)
All input DMAs are issued on the sync ring in strict priority order
(wt, x g0, pf32, x g1, cg, x g2, x g3) so the first matmul's operands
land first; stores go on the otherwise-idle gpsimd ring.
"""
import numpy as np
import ml_dtypes
from contextlib import ExitStack

import concourse.bass as bass
import concourse.tile as tile
from concourse import bacc, mybir

INPUT_DIM = 512
N_CLASSES = 512
N_TREES = 64
N_LEAVES = 8
N_INTERNAL = 7
PPT = N_INTERNAL * (INPUT_DIM + 1) + N_LEAVES * N_CLASSES
BATCH = 16384
N_CORES = 8
BSH = BATCH // N_CORES          # 2048 rows per core
NG = 4                          # 4 groups of 512 rows
NW = N_INTERNAL * INPUT_DIM

F32 = mybir.dt.float32
BF16 = mybir.dt.bfloat16


def _emit(ctx: ExitStack, tc, xt, pbf, pf32, out):
    nc = tc.nc
    AF = mybir.ActivationFunctionType
    MUL = mybir.AluOpType.mult
    ADD = mybir.AluOpType.add

    const = ctx.enter_context(tc.tile_pool(name="const", bufs=1))

    # Packed bf16 params, resident in SBUF:
    #   [:, k*256 + jb*128 + j'] = W^T chunk (d = k*128+p, j = jb*128+j')
    #   [:, 1024:1536] = cg0 rows i*64+t (i=0,1), [:, 1536:2048] = cg1 (i=2,3')
    pb = const.tile([128, 2048], BF16)
    cg0 = pb[:, 1024:1536]
    cg1 = pb[:, 1536:2048]
    pf = const.tile([128, 3], F32)
    # x^T resident in SBUF (pre-transposed per group on host):
    #   xT[p, g, k*512 + b] = x[g*512 + b, k*128 + p]
    xT = const.tile([128, NG, 2048], BF16)

    # Input DMAs: one ring (sync/HWDGE), strict FIFO = priority order.
    nc.sync.dma_start(pb[:, 0:1024], pbf[:, 0:1024])                  # wt
    nc.sync.dma_start(xT[:, 0, 0:1024], xt[0:128, 0:1024])            # g0 k01
    nc.sync.dma_start(xT[:, 0, 1024:2048], xt[0:128, 1024:2048])      # g0 k23
    nc.sync.dma_start(pf[:], pf32[:])
    nc.sync.dma_start(xT[:, 1, :], xt[128:256, :])                    # g1
    nc.sync.dma_start(pb[:, 1024:2048], pbf[:, 1024:2048])            # cg
    nc.sync.dma_start(xT[:, 2, :], xt[256:384, :])                    # g2
    nc.sync.dma_start(xT[:, 3, :], xt[384:512, :])                    # g3

    dpool = ctx.enter_context(tc.tile_pool(name="work", bufs=2))
    epool = ctx.enter_context(tc.tile_pool(name="eT", bufs=2))
    opool = ctx.enter_context(tc.tile_pool(name="osb", bufs=2))
    zpp = ctx.enter_context(tc.tile_pool(name="zps", bufs=2, space="PSUM"))
    opp = ctx.enter_context(tc.tile_pool(name="ops", bufs=2, space="PSUM"))

    def mm1(g):
        # zT[j, b] = sum_d W[j, d] x[b, d]    j = i*64 + t, i-major
        zt0 = zpp.tile([128, 512], F32, tag="zt0")
        zt1 = zpp.tile([128, 512], F32, tag="zt1")
        for jb, zt in ((0, zt0), (1, zt1)):
            for k in range(4):
                nc.tensor.matmul(zt[:],
                                 lhsT=pb[:, k * 256 + jb * 128:
                                         k * 256 + (jb + 1) * 128],
                                 rhs=xT[:, g, k * 512:(k + 1) * 512],
                                 start=(k == 0), stop=(k == 3))
        return zt0, zt1

    zts = mm1(0)
    for g in range(NG):
        zt0, zt1 = zts
        d0 = dpool.tile([128, 512], BF16, tag="d0")
        nc.scalar.activation(d0[:], zt0[:], AF.Sigmoid, bias=pf[:, 0:1])
        # r = 1/(4 + d3) = 1/4 - sigmoid(z3 + ln 1.25)/20   (exact identity)
        s3 = dpool.tile([128, 512], F32, tag="s3")
        nc.scalar.activation(s3[64:128, :], zt1[64:128, :], AF.Sigmoid,
                             bias=pf[64:128, 2:3])
        rf = dpool.tile([128, 512], BF16, tag="rf")
        nc.vector.tensor_scalar(rf[64:128, :], s3[64:128, :], -0.05, 0.25,
                                op0=MUL, op1=ADD)
        nc.vector.tensor_scalar(rf[0:64, :], s3[64:128, :], -0.05, 0.25,
                                op0=MUL, op1=ADD)       # partition-shifted
        d1 = dpool.tile([128, 512], BF16, tag="d1")
        nc.scalar.activation(d1[:], zt1[:], AF.Sigmoid, bias=pf[:, 1:2])

        # e = d * r  -> lhsT tiles for mm2 (rows i*64+t match cg rows)
        T0 = epool.tile([128, 512], BF16, tag="T0")
        T1 = epool.tile([128, 512], BF16, tag="T1")
        nc.vector.tensor_tensor(T0[:], d0[:], rf[:], op=MUL)
        nc.vector.tensor_tensor(T1[:], d1[:], rf[:], op=MUL)

        # keep the PE fed: next group's mm1 goes ahead of this group's mm2
        if g + 1 < NG:
            zts = mm1(g + 1)

        # out = T0 @ cg0 + T1 @ cg1  per batch tile; evacuate in halves
        osb = opool.tile([128, 4, 512], BF16, tag="osb")
        for h in range(2):
            ops = opp.tile([128, 2, 512], F32, tag="o")
            for bt in (2 * h, 2 * h + 1):
                bs = slice(bt * 128, (bt + 1) * 128)
                nc.tensor.matmul(ops[:, bt - 2 * h, :], lhsT=T0[:, bs],
                                 rhs=cg0, start=True, stop=False)
                nc.tensor.matmul(ops[:, bt - 2 * h, :], lhsT=T1[:, bs],
                                 rhs=cg1, start=False, stop=True)
            nc.vector.tensor_copy(osb[:, 2 * h:2 * h + 2, :], ops[:])
        # out row = g*128 + p, col = bt*512 + c  (host un-permutes)
        nc.gpsimd.dma_start(out[g * 128:(g + 1) * 128, :],
                            osb[:].rearrange("p bt c -> p (bt c)"))


_NC = None
_RUNNER = None


def _get_nc():
    global _NC
    if _NC is None:
        nc = bacc.Bacc("TRN2", target_bir_lowering=False, debug=False)
        xt = nc.dram_tensor("xt", [512, 2048], BF16, kind="ExternalInput")
        pbf = nc.dram_tensor("pbf", [128, 2048], BF16, kind="ExternalInput")
        pf32 = nc.dram_tensor("pf32", [128, 3], F32, kind="ExternalInput")
        out = nc.dram_tensor("out", [512, 2048], BF16, kind="ExternalOutput")
        with tile.TileContext(nc) as tc, ExitStack() as ctx:
            _emit(ctx, tc, xt.ap(), pbf.ap(), pf32.ap(), out.ap())
        nc.compile()
        _NC = nc
    return _NC


def _get_runner():
    """Build the sharded PJRT executable ONCE (jit + NEFF compile are cached
    across kernel() calls; run_bass_kernel_spmd would re-trace every call)."""
    global _RUNNER
    if _RUNNER is None:
        import jax
        import jax.numpy as jnp
        from jax.sharding import Mesh, PartitionSpec, NamedSharding
        from jax.experimental.shard_map import shard_map
        from concourse import bass2jax

        nc = _get_nc()
        bass2jax.install_neuronx_cc_hook()

        part_name = (nc.partition_id_tensor.name
                     if nc.partition_id_tensor else None)
        in_names, out_names, out_avals = [], [], []
        for alloc in nc.m.functions[0].allocations:
            if not isinstance(alloc, mybir.MemoryLocationSet):
                continue
            name = alloc.memorylocations[0].name
            if alloc.kind == "ExternalInput":
                if name != part_name:
                    in_names.append(name)
            elif alloc.kind == "ExternalOutput":
                out_names.append(name)
                out_avals.append(jax.core.ShapedArray(
                    tuple(alloc.tensor_shape), mybir.dt.np(alloc.dtype)))
        n_params = len(in_names)
        all_names = tuple(in_names) + tuple(out_names)
        if part_name is not None:
            all_names = all_names + (part_name,)
        donate = tuple(range(n_params, n_params + len(out_names)))

        def _body(*args):
            operands = list(args)
            if part_name is not None:
                operands.append(bass2jax.partition_id_tensor())
            outs = bass2jax._bass_exec_p.bind(
                *operands,
                out_avals=tuple(out_avals),
                in_names=all_names,
                out_names=tuple(out_names),
                lowering_input_output_aliases=(),
                sim_require_finite=True,
                sim_require_nnan=True,
                nc=nc,
            )
            return tuple(outs)

        devices = jax.devices()[:N_CORES]
        mesh = Mesh(np.asarray(devices), ("core",))
        spec = PartitionSpec("core")
        fn = jax.jit(
            shard_map(_body, mesh=mesh,
                      in_specs=(spec,) * (n_params + len(out_names)),
                      out_specs=(spec,) * len(out_names), check_rep=False),
            donate_argnums=donate, keep_unused=True)
        zmk = jax.jit(
            lambda: jnp.zeros((N_CORES * 512, 2048), ml_dtypes.bfloat16),
            out_shardings=NamedSharding(mesh, spec))
        _RUNNER = (fn, zmk, in_names)
    return _RUNNER


def _host_prep(x, tree_params, tree_weights):
    """Host-side: transpose/group x, pack replicated params, and fold the
    leaf-distribution combination matrices (incl. softmax) plus the rank-1
    output shift S into precomputed arrays."""
    x = np.asarray(x, np.float32).astype(ml_dtypes.bfloat16)
    # xt[(g p), (k b)] = x_core[g*512 + b, k*128 + p], per core
    xt = np.ascontiguousarray(
        x.reshape(N_CORES, NG, 512, 4, 128).transpose(0, 1, 4, 3, 2)
    ).reshape(N_CORES * 512, 2048)

    p = np.asarray(tree_params, np.float32)[0].reshape(N_TREES, PPT)
    w4 = p[:, :NW].reshape(N_TREES, N_INTERNAL, INPUT_DIM)[:, :4, :]
    wj = w4.transpose(1, 0, 2).reshape(256, INPUT_DIM)      # j = i*64 + t
    # wt packed [128, 4k, 256j] -> [128, 1024]
    wt = np.ascontiguousarray(
        wj.T.reshape(4, 128, 256).transpose(1, 0, 2)).reshape(128, 1024)

    ll = p[:, NW + N_INTERNAL:].reshape(N_TREES, N_LEAVES, N_CLASSES)
    e = np.exp(ll - ll.max(axis=-1, keepdims=True))
    M = e / e.sum(axis=-1, keepdims=True)                   # softmax [T, L, C]
    M = M * np.asarray(tree_weights, np.float32)[0][:, None, None]
    C_ = M[:, 0] + M[:, 2] + M[:, 4] + M[:, 6]              # [T, C]
    G0 = M[:, 1] - M[:, 2]
    G1 = M[:, 3] - M[:, 4]
    G2 = M[:, 5] - M[:, 6]
    G3 = M[:, 7] - C_ * 0.25
    cg0 = np.concatenate([G0, G1], 0)                       # [128, C]
    cg1 = np.concatenate([G2, G3], 0)
    pbf = np.concatenate(
        [wt, cg0.astype(np.float32), cg1.astype(np.float32)],
        axis=1).astype(ml_dtypes.bfloat16)                  # [128, 2048]

    bias = p[:, NW:NW + N_INTERNAL][:, :4].T.reshape(256)   # j-major
    pf32 = np.zeros((128, 3), np.float32)
    pf32[:, 0] = bias[0:128]
    pf32[:, 1] = bias[128:256]
    pf32[64:128, 2] = bias[192:256] + np.float32(np.log(1.25))

    S = C_.sum(axis=0) * 0.25                               # [C] host shift
    return xt, pbf, pf32, S


def _unpermute(outd, S):
    """outd [N_CORES*512, 2048] with row g*128+p, col bt*512+c ->
    full [16384, 512] f32 plus the rank-1 shift."""
    o = outd.reshape(N_CORES, NG, 128, 4, 512).transpose(0, 1, 3, 2, 4)
    return np.ascontiguousarray(o).reshape(BATCH, N_CLASSES).astype(
        np.float32) + S[None, :]


def kernel(x: np.ndarray, tree_params: np.ndarray,
           tree_weights: np.ndarray) -> np.ndarray:
    fn, zmk, in_names = _get_runner()
    xt, pbf, pf32, S = _host_prep(x, tree_params, tree_weights)
    reps = {"xt": xt,
            "pbf": np.concatenate([pbf] * N_CORES, 0),
            "pf32": np.concatenate([pf32] * N_CORES, 0)}
    args = [reps[n] for n in in_names] + [zmk()]
    outs = fn(*args)
    return _unpermute(np.asarray(outs[0]), S)


# revision 7
# speedup vs baseline: 1.6858x; 1.0495x over previous
"""Data-parallel GeneratedTreeClassifier forward on 8 NeuronCores (Bass/Tile).

Shards the batch dim of x (16384 -> 8 x 2048) across cores, replicates the
small tree params, runs a hand-written Bass/Tile kernel per core, and
gathers the full [16384, 512] output.

Math restructure (per tree t, decisions i = 0..3, r = 1/(4 + d3)):
  leaf_probs = r * [1, d0, 1-d0, d1, 1-d1, d2, 1-d2, d3]
  out = sum_t  r C_t + sum_{i<3} (r d_i) G_ti + (r d3) M_t7
  with  e_i = d_i r  and the exact identity  e3 = 1 - 4 r:
  out = S[c] + e @ [G0; G1; G2; G3']      (K = 256)
  G3' = M_t7 - C_t/4,  S[c] = sum_t C_t[c]/4   (rank-1, added on HOST)
  where M = softmax(leaf_logits) * w_tree; C/G row-combines of M are all
  precomputed on the host (cg0 = [G0;G1], cg1 = [G2;G3']).

Per-core device graph (4 groups of 512 batch rows, software-pipelined so
mm1 of group g+1 runs on the PE while group g's ACT/DVE chain computes):
  zT   = W @ x^T                 (PE, 8 matmuls N=512; x^T staged on host)
  d    = sigmoid(zT + bias)      (ACT, bias per-partition)
  r    = 1/4 - sigmoid(z3 + ln1.25)/20    (exact identity; the two rf
         halves are written by partition-shifted DVE tensor_scalar ops,
         no broadcast DMA)
  T0   = d01 * rf, T1 = d23 * rf (DVE, bf16)
  out  = T0 @ cg0 + T1 @ cg1     (PE, 2 matmuls N=512 per batch tile)
All input DMAs are issued on the sync ring in strict priority order
(wt, x g0, pf32, x g1, cg, x g2, x g3) so the first matmul's operands
land first; stores go on the otherwise-idle gpsimd ring.
"""
import numpy as np
import ml_dtypes
from contextlib import ExitStack

import concourse.bass as bass
import concourse.tile as tile
from concourse import bacc, mybir

INPUT_DIM = 512
N_CLASSES = 512
N_TREES = 64
N_LEAVES = 8
N_INTERNAL = 7
PPT = N_INTERNAL * (INPUT_DIM + 1) + N_LEAVES * N_CLASSES
BATCH = 16384
N_CORES = 8
BSH = BATCH // N_CORES          # 2048 rows per core
NG = 4                          # 4 groups of 512 rows
NW = N_INTERNAL * INPUT_DIM

F32 = mybir.dt.float32
BF16 = mybir.dt.bfloat16


def _emit(ctx: ExitStack, tc, xt, pbf, pf32, out):
    nc = tc.nc
    AF = mybir.ActivationFunctionType
    MUL = mybir.AluOpType.mult
    ADD = mybir.AluOpType.add

    const = ctx.enter_context(tc.tile_pool(name="const", bufs=1))

    # Packed bf16 params, resident in SBUF:
    #   [:, k*256 + jb*128 + j'] = W^T chunk (d = k*128+p, j = jb*128+j')
    #   [:, 1024:1536] = cg0 rows i*64+t (i=0,1), [:, 1536:2048] = cg1 (i=2,3')
    pb = const.tile([128, 2048], BF16)
    cg0 = pb[:, 1024:1536]
    cg1 = pb[:, 1536:2048]
    pf = const.tile([128, 3], F32)
    # x^T resident in SBUF (pre-transposed per group on host):
    #   xT[p, g, k*512 + b] = x[g*512 + b, k*128 + p]
    xT = const.tile([128, NG, 2048], BF16)

    # Input DMAs: one ring (sync/HWDGE), strict FIFO = priority order.
    nc.sync.dma_start(pb[:, 0:1024], pbf[:, 0:1024])                  # wt
    nc.sync.dma_start(xT[:, 0, 0:1024], xt[0:128, 0:1024])            # g0 k01
    nc.sync.dma_start(xT[:, 0, 1024:2048], xt[0:128, 1024:2048])      # g0 k23
    nc.sync.dma_start(pf[:], pf32[:])
    nc.sync.dma_start(xT[:, 1, :], xt[128:256, :])                    # g1
    nc.sync.dma_start(pb[:, 1024:2048], pbf[:, 1024:2048])            # cg
    nc.sync.dma_start(xT[:, 2, :], xt[256:384, :])                    # g2
    nc.sync.dma_start(xT[:, 3, :], xt[384:512, :])                    # g3

    dpool = ctx.enter_context(tc.tile_pool(name="work", bufs=2))
    epool = ctx.enter_context(tc.tile_pool(name="eT", bufs=2))
    opool = ctx.enter_context(tc.tile_pool(name="osb", bufs=2))
    zpp = ctx.enter_context(tc.tile_pool(name="zps", bufs=2, space="PSUM"))
    opp = ctx.enter_context(tc.tile_pool(name="ops", bufs=2, space="PSUM"))

    # Warm the PE HAM clock-gate while the input DMAs are in flight: a few
    # dependency-free matmuls on zeroed scratch, written into group 0's zt0
    # tile (the first real matmul restarts the accumulation with start=True).
    scratch = const.tile([128, 512], BF16)
    nc.gpsimd.memset(scratch[:], 0.0)
    warm = zpp.tile([128, 512], F32, tag="zt0")
    for _ in range(6):
        nc.tensor.matmul(warm[:], lhsT=scratch[:, 0:128], rhs=scratch[:],
                         start=True, stop=True)

    def mm1(g, zt0=None):
        # zT[j, b] = sum_d W[j, d] x[b, d]    j = i*64 + t, i-major
        if zt0 is None:
            zt0 = zpp.tile([128, 512], F32, tag="zt0")
        zt1 = zpp.tile([128, 512], F32, tag="zt1")
        for jb, zt in ((0, zt0), (1, zt1)):
            for k in range(4):
                nc.tensor.matmul(zt[:],
                                 lhsT=pb[:, k * 256 + jb * 128:
                                         k * 256 + (jb + 1) * 128],
                                 rhs=xT[:, g, k * 512:(k + 1) * 512],
                                 start=(k == 0), stop=(k == 3))
        return zt0, zt1

    zts = mm1(0, zt0=warm)
    for g in range(NG):
        zt0, zt1 = zts
        d0 = dpool.tile([128, 512], BF16, tag="d0")
        nc.scalar.activation(d0[:], zt0[:], AF.Sigmoid, bias=pf[:, 0:1])
        # r = 1/(4 + d3) = 1/4 - sigmoid(z3 + ln 1.25)/20   (exact identity)
        s3 = dpool.tile([128, 512], F32, tag="s3")
        nc.scalar.activation(s3[64:128, :], zt1[64:128, :], AF.Sigmoid,
                             bias=pf[64:128, 2:3])
        rf = dpool.tile([128, 512], BF16, tag="rf")
        nc.vector.tensor_scalar(rf[64:128, :], s3[64:128, :], -0.05, 0.25,
                                op0=MUL, op1=ADD)
        nc.vector.tensor_scalar(rf[0:64, :], s3[64:128, :], -0.05, 0.25,
                                op0=MUL, op1=ADD)       # partition-shifted
        d1 = dpool.tile([128, 512], BF16, tag="d1")
        nc.scalar.activation(d1[:], zt1[:], AF.Sigmoid, bias=pf[:, 1:2])

        # e = d * r  -> lhsT tiles for mm2 (rows i*64+t match cg rows)
        T0 = epool.tile([128, 512], BF16, tag="T0")
        T1 = epool.tile([128, 512], BF16, tag="T1")
        nc.vector.tensor_tensor(T0[:], d0[:], rf[:], op=MUL)
        nc.vector.tensor_tensor(T1[:], d1[:], rf[:], op=MUL)

        # keep the PE fed: next group's mm1 goes ahead of this group's mm2
        if g + 1 < NG:
            zts = mm1(g + 1)

        # out = T0 @ cg0 + T1 @ cg1  per batch tile; evacuate in halves
        # (one half on DVE, one on ACT to balance engine load)
        osb = opool.tile([128, 4, 512], BF16, tag="osb")
        for h in range(2):
            ops = opp.tile([128, 2, 512], F32, tag="o")
            for bt in (2 * h, 2 * h + 1):
                bs = slice(bt * 128, (bt + 1) * 128)
                nc.tensor.matmul(ops[:, bt - 2 * h, :], lhsT=T0[:, bs],
                                 rhs=cg0, start=True, stop=False)
                nc.tensor.matmul(ops[:, bt - 2 * h, :], lhsT=T1[:, bs],
                                 rhs=cg1, start=False, stop=True)
            if h == 0:
                nc.vector.tensor_copy(osb[:, 0:2, :], ops[:])
            else:
                nc.scalar.copy(osb[:, 2:4, :], ops[:])
            # out row = g*128 + p, col = bt*512 + c  (host un-permutes);
            # the last group stores per half so the final bytes leave early
            if g == NG - 1:
                nc.gpsimd.dma_start(
                    out[g * 128:(g + 1) * 128, h * 1024:(h + 1) * 1024],
                    osb[:, 2 * h:2 * h + 2, :].rearrange("p bt c -> p (bt c)"))
        if g < NG - 1:
            nc.gpsimd.dma_start(out[g * 128:(g + 1) * 128, :],
                                osb[:].rearrange("p bt c -> p (bt c)"))


_NC = None
_RUNNER = None


def _get_nc():
    global _NC
    if _NC is None:
        nc = bacc.Bacc("TRN2", target_bir_lowering=False, debug=False)
        xt = nc.dram_tensor("xt", [512, 2048], BF16, kind="ExternalInput")
        pbf = nc.dram_tensor("pbf", [128, 2048], BF16, kind="ExternalInput")
        pf32 = nc.dram_tensor("pf32", [128, 3], F32, kind="ExternalInput")
        out = nc.dram_tensor("out", [512, 2048], BF16, kind="ExternalOutput")
        with tile.TileContext(nc) as tc, ExitStack() as ctx:
            _emit(ctx, tc, xt.ap(), pbf.ap(), pf32.ap(), out.ap())
        nc.compile()
        _NC = nc
    return _NC


def _get_runner():
    """Build the sharded PJRT executable ONCE (jit + NEFF compile are cached
    across kernel() calls; run_bass_kernel_spmd would re-trace every call)."""
    global _RUNNER
    if _RUNNER is None:
        import jax
        import jax.numpy as jnp
        from jax.sharding import Mesh, PartitionSpec, NamedSharding
        from jax.experimental.shard_map import shard_map
        from concourse import bass2jax

        nc = _get_nc()
        bass2jax.install_neuronx_cc_hook()

        part_name = (nc.partition_id_tensor.name
                     if nc.partition_id_tensor else None)
        in_names, out_names, out_avals = [], [], []
        for alloc in nc.m.functions[0].allocations:
            if not isinstance(alloc, mybir.MemoryLocationSet):
                continue
            name = alloc.memorylocations[0].name
            if alloc.kind == "ExternalInput":
                if name != part_name:
                    in_names.append(name)
            elif alloc.kind == "ExternalOutput":
                out_names.append(name)
                out_avals.append(jax.core.ShapedArray(
                    tuple(alloc.tensor_shape), mybir.dt.np(alloc.dtype)))
        n_params = len(in_names)
        all_names = tuple(in_names) + tuple(out_names)
        if part_name is not None:
            all_names = all_names + (part_name,)
        donate = tuple(range(n_params, n_params + len(out_names)))

        def _body(*args):
            operands = list(args)
            if part_name is not None:
                operands.append(bass2jax.partition_id_tensor())
            outs = bass2jax._bass_exec_p.bind(
                *operands,
                out_avals=tuple(out_avals),
                in_names=all_names,
                out_names=tuple(out_names),
                lowering_input_output_aliases=(),
                sim_require_finite=True,
                sim_require_nnan=True,
                nc=nc,
            )
            return tuple(outs)

        devices = jax.devices()[:N_CORES]
        mesh = Mesh(np.asarray(devices), ("core",))
        spec = PartitionSpec("core")
        fn = jax.jit(
            shard_map(_body, mesh=mesh,
                      in_specs=(spec,) * (n_params + len(out_names)),
                      out_specs=(spec,) * len(out_names), check_rep=False),
            donate_argnums=donate, keep_unused=True)
        zmk = jax.jit(
            lambda: jnp.zeros((N_CORES * 512, 2048), ml_dtypes.bfloat16),
            out_shardings=NamedSharding(mesh, spec))
        _RUNNER = (fn, zmk, in_names)
    return _RUNNER


def _host_prep(x, tree_params, tree_weights):
    """Host-side: transpose/group x, pack replicated params, and fold the
    leaf-distribution combination matrices (incl. softmax) plus the rank-1
    output shift S into precomputed arrays."""
    x = np.asarray(x, np.float32).astype(ml_dtypes.bfloat16)
    # xt[(g p), (k b)] = x_core[g*512 + b, k*128 + p], per core
    xt = np.ascontiguousarray(
        x.reshape(N_CORES, NG, 512, 4, 128).transpose(0, 1, 4, 3, 2)
    ).reshape(N_CORES * 512, 2048)

    p = np.asarray(tree_params, np.float32)[0].reshape(N_TREES, PPT)
    w4 = p[:, :NW].reshape(N_TREES, N_INTERNAL, INPUT_DIM)[:, :4, :]
    wj = w4.transpose(1, 0, 2).reshape(256, INPUT_DIM)      # j = i*64 + t
    # wt packed [128, 4k, 256j] -> [128, 1024]
    wt = np.ascontiguousarray(
        wj.T.reshape(4, 128, 256).transpose(1, 0, 2)).reshape(128, 1024)

    ll = p[:, NW + N_INTERNAL:].reshape(N_TREES, N_LEAVES, N_CLASSES)
    e = np.exp(ll - ll.max(axis=-1, keepdims=True))
    M = e / e.sum(axis=-1, keepdims=True)                   # softmax [T, L, C]
    M = M * np.asarray(tree_weights, np.float32)[0][:, None, None]
    C_ = M[:, 0] + M[:, 2] + M[:, 4] + M[:, 6]              # [T, C]
    G0 = M[:, 1] - M[:, 2]
    G1 = M[:, 3] - M[:, 4]
    G2 = M[:, 5] - M[:, 6]
    G3 = M[:, 7] - C_ * 0.25
    cg0 = np.concatenate([G0, G1], 0)                       # [128, C]
    cg1 = np.concatenate([G2, G3], 0)
    pbf = np.concatenate(
        [wt, cg0.astype(np.float32), cg1.astype(np.float32)],
        axis=1).astype(ml_dtypes.bfloat16)                  # [128, 2048]

    bias = p[:, NW:NW + N_INTERNAL][:, :4].T.reshape(256)   # j-major
    pf32 = np.zeros((128, 3), np.float32)
    pf32[:, 0] = bias[0:128]
    pf32[:, 1] = bias[128:256]
    pf32[64:128, 2] = bias[192:256] + np.float32(np.log(1.25))

    S = C_.sum(axis=0) * 0.25                               # [C] host shift
    return xt, pbf, pf32, S


def _unpermute(outd, S):
    """outd [N_CORES*512, 2048] with row g*128+p, col bt*512+c ->
    full [16384, 512] f32 plus the rank-1 shift."""
    o = outd.reshape(N_CORES, NG, 128, 4, 512).transpose(0, 1, 3, 2, 4)
    return np.ascontiguousarray(o).reshape(BATCH, N_CLASSES).astype(
        np.float32) + S[None, :]


def kernel(x: np.ndarray, tree_params: np.ndarray,
           tree_weights: np.ndarray) -> np.ndarray:
    fn, zmk, in_names = _get_runner()
    xt, pbf, pf32, S = _host_prep(x, tree_params, tree_weights)
    reps = {"xt": xt,
            "pbf": np.concatenate([pbf] * N_CORES, 0),
            "pf32": np.concatenate([pf32] * N_CORES, 0)}
    args = [reps[n] for n in in_names] + [zmk()]
    outs = fn(*args)
    return _unpermute(np.asarray(outs[0]), S)


# revision 8
# speedup vs baseline: 1.7224x; 1.0217x over previous
"""Data-parallel GeneratedTreeClassifier forward on 8 NeuronCores (Bass/Tile).

Shards the batch dim of x (16384 -> 8 x 2048) across cores, replicates the
small tree params, runs a hand-written Bass/Tile kernel per core, and
gathers the full [16384, 512] output.

Math restructure (per tree t, decisions i = 0..3, r = 1/(4 + d3)):
  leaf_probs = r * [1, d0, 1-d0, d1, 1-d1, d2, 1-d2, d3]
  out = S[c] + e @ [G0; G1; G2; G3']      (K = 256, e_i = d_i r)
  G3' = M_t7 - C_t/4,  S[c] = sum_t C_t[c]/4   (rank-1, added on HOST)
  where M = softmax(leaf_logits) * w_tree; C/G row-combines of M are all
  precomputed on the host (cg = [G0;G1;G2;G3']).

Both matmuls run in fp8e4m3 with perf_mode=DoubleRow (K packed 2/cell), so
mm1 (K=512) is 4 matmuls and mm2 (K=256) is 4 matmuls per 512-row group.
fp8 scaling: T tiles hold 4e (rf4 = 1 - 0.2*sigmoid = 4r), cg is scaled
x16 on the host, and the PSUM evacuation copies divide by 64.

Per-core device graph (4 groups of 512 batch rows, software-pipelined so
mm1 of group g+1 runs on the PE while group g's ACT/DVE chain computes):
  zT   = W @ x^T                 (PE, DoubleRow fp8)
  d    = sigmoid(zT + bias)      (ACT, bias per-partition)
  rf4  = 1 - 0.2*sigmoid(z3 + ln1.25)   (exact: = 4/(4+d3); both halves
         written by partition-shifted DVE tensor_scalar ops)
  Td   = [d01; d23] * rf4        (DVE, fp8 out, lhsT of mm2)
  out  = Td @ cgd / 64           (PE DoubleRow; evacuate halves DVE/ACT)
All input DMAs are issued on the sync ring in strict priority order; the
PE HAM clock-gate is pre-warmed with dummy matmuls during the DMA wait.
"""
import numpy as np
import ml_dtypes
from contextlib import ExitStack

import concourse.bass as bass
import concourse.tile as tile
from concourse import bacc, mybir

INPUT_DIM = 512
N_CLASSES = 512
N_TREES = 64
N_LEAVES = 8
N_INTERNAL = 7
PPT = N_INTERNAL * (INPUT_DIM + 1) + N_LEAVES * N_CLASSES
BATCH = 16384
N_CORES = 8
BSH = BATCH // N_CORES          # 2048 rows per core
NG = 4                          # 4 groups of 512 rows
NW = N_INTERNAL * INPUT_DIM

F32 = mybir.dt.float32
BF16 = mybir.dt.bfloat16
F8 = mybir.dt.float8e4
F8NP = ml_dtypes.float8_e4m3
DR = mybir.MatmulPerfMode.DoubleRow


def _emit(ctx: ExitStack, tc, xt, pbf, pf32, out):
    nc = tc.nc
    AF = mybir.ActivationFunctionType
    MUL = mybir.AluOpType.mult
    ADD = mybir.AluOpType.add

    const = ctx.enter_context(tc.tile_pool(name="const", bufs=1))

    # fp8 params: wt8[p, kk, ko, j] = W^T[kk*256+ko*128+p, j]  (j = i*64+t)
    wt8 = const.tile([128, 2, 2, 256], F8)
    # cgd[p, ko, c] = 16 * cg rows (ko*128+p)
    cgd = const.tile([128, 2, 512], F8)
    pf = const.tile([128, 3], F32)
    # x^T fp8: xT[p, g, kk, ko, b] = x[g*512 + b, kk*256 + ko*128 + p]
    xT = const.tile([128, NG, 2, 2, 512], F8)

    # Input DMAs: one ring (sync/HWDGE), strict FIFO = priority order.
    nc.sync.dma_start(wt8[:], pbf[:, 0:1024].rearrange(
        "p (kk ko j) -> p kk ko j", kk=2, ko=2))
    nc.sync.dma_start(xT[:, 0], xt[0:128, :].rearrange(
        "p (kk ko b) -> p kk ko b", kk=2, ko=2))
    nc.sync.dma_start(pf[:], pf32[:])
    nc.sync.dma_start(xT[:, 1], xt[128:256, :].rearrange(
        "p (kk ko b) -> p kk ko b", kk=2, ko=2))
    nc.sync.dma_start(cgd[:], pbf[:, 1024:2048].rearrange(
        "p (ko c) -> p ko c", ko=2))
    nc.sync.dma_start(xT[:, 2], xt[256:384, :].rearrange(
        "p (kk ko b) -> p kk ko b", kk=2, ko=2))
    nc.sync.dma_start(xT[:, 3], xt[384:512, :].rearrange(
        "p (kk ko b) -> p kk ko b", kk=2, ko=2))

    dpool = ctx.enter_context(tc.tile_pool(name="work", bufs=2))
    epool = ctx.enter_context(tc.tile_pool(name="eT", bufs=2))
    opool = ctx.enter_context(tc.tile_pool(name="osb", bufs=2))
    zpp = ctx.enter_context(tc.tile_pool(name="zps", bufs=2, space="PSUM"))
    opp = ctx.enter_context(tc.tile_pool(name="ops", bufs=2, space="PSUM"))

    # Warm the PE HAM clock-gate while the input DMAs are in flight: a few
    # dependency-free matmuls on zeroed scratch, written into group 0's zt
    # tile (the first real matmul restarts the accumulation with start=True).
    scratch = const.tile([128, 512], BF16)
    nc.gpsimd.memset(scratch[:], 0.0)
    warm = zpp.tile([128, 2, 512], F32, tag="zt")
    for _ in range(8):
        nc.tensor.matmul(warm[:, 0, :], lhsT=scratch[:, 0:128], rhs=scratch[:],
                         start=True, stop=True)

    def mm1(g, zt=None):
        # zT[j, b] = sum_d W[j, d] x[b, d]    j = i*64 + t, i-major
        if zt is None:
            zt = zpp.tile([128, 2, 512], F32, tag="zt")
        for jb in range(2):
            for kk in range(2):
                nc.tensor.matmul(zt[:, jb, :],
                                 lhsT=wt8[:, kk, :, jb * 128:(jb + 1) * 128],
                                 rhs=xT[:, g, kk],
                                 perf_mode=DR,
                                 start=(kk == 0), stop=(kk == 1))
        return zt

    zt = mm1(0, zt=warm)
    for g in range(NG):
        d0 = dpool.tile([128, 512], BF16, tag="d0")
        nc.scalar.activation(d0[:], zt[:, 0, :], AF.Sigmoid, bias=pf[:, 0:1])
        # rf4 = 4/(4 + d3) = 1 - 0.8*sigmoid(z3 + ln 1.25)/... exact identity
        s3 = dpool.tile([128, 512], F32, tag="s3")
        nc.scalar.activation(s3[64:128, :], zt[64:128, 1, :], AF.Sigmoid,
                             bias=pf[64:128, 2:3])
        rf = dpool.tile([128, 512], BF16, tag="rf")
        nc.vector.tensor_scalar(rf[64:128, :], s3[64:128, :], -0.2, 1.0,
                                op0=MUL, op1=ADD)
        nc.vector.tensor_scalar(rf[0:64, :], s3[64:128, :], -0.2, 1.0,
                                op0=MUL, op1=ADD)       # partition-shifted
        d1 = dpool.tile([128, 512], BF16, tag="d1")
        nc.scalar.activation(d1[:], zt[:, 1, :], AF.Sigmoid, bias=pf[:, 1:2])

        # Td[:, ko, :] = 4 * e rows (ko*128+p)  -> fp8 lhsT for mm2
        Td = epool.tile([128, 2, 512], F8, tag="Td")
        nc.vector.tensor_tensor(Td[:, 0, :], d0[:], rf[:], op=MUL)
        nc.vector.tensor_tensor(Td[:, 1, :], d1[:], rf[:], op=MUL)

        # keep the PE fed: next group's mm1 goes ahead of this group's mm2
        if g + 1 < NG:
            zt = mm1(g + 1)

        # out = Td @ cgd / 64  per batch tile; evacuate in halves (DVE/ACT)
        osb = opool.tile([128, 4, 512], BF16, tag="osb")
        for h in range(2):
            ops = opp.tile([128, 2, 512], F32, tag="o")
            for bt in (2 * h, 2 * h + 1):
                bs = slice(bt * 128, (bt + 1) * 128)
                nc.tensor.matmul(ops[:, bt - 2 * h, :],
                                 lhsT=Td[:, :, bs], rhs=cgd[:],
                                 perf_mode=DR, start=True, stop=True)
            if h == 0:
                nc.vector.tensor_scalar(osb[:, 0:2, :], ops[:],
                                        1.0 / 64.0, 0.0, op0=MUL, op1=ADD)
            else:
                nc.scalar.mul(osb[:, 2:4, :], ops[:], 1.0 / 64.0)
            # out row = g*128 + p, col = bt*512 + c  (host un-permutes);
            # the last group stores per half so the final bytes leave early
            if g == NG - 1:
                nc.gpsimd.dma_start(
                    out[g * 128:(g + 1) * 128, h * 1024:(h + 1) * 1024],
                    osb[:, 2 * h:2 * h + 2, :].rearrange("p bt c -> p (bt c)"))
        if g < NG - 1:
            nc.gpsimd.dma_start(out[g * 128:(g + 1) * 128, :],
                                osb[:].rearrange("p bt c -> p (bt c)"))


_NC = None
_RUNNER = None


def _get_nc():
    global _NC
    if _NC is None:
        nc = bacc.Bacc("TRN2", target_bir_lowering=False, debug=False)
        xt = nc.dram_tensor("xt", [512, 2048], F8, kind="ExternalInput")
        pbf = nc.dram_tensor("pbf", [128, 2048], F8, kind="ExternalInput")
        pf32 = nc.dram_tensor("pf32", [128, 3], F32, kind="ExternalInput")
        out = nc.dram_tensor("out", [512, 2048], BF16, kind="ExternalOutput")
        with tile.TileContext(nc) as tc, ExitStack() as ctx:
            _emit(ctx, tc, xt.ap(), pbf.ap(), pf32.ap(), out.ap())
        nc.compile()
        _NC = nc
    return _NC


def _get_runner():
    """Build the sharded PJRT executable ONCE (jit + NEFF compile are cached
    across kernel() calls; run_bass_kernel_spmd would re-trace every call)."""
    global _RUNNER
    if _RUNNER is None:
        import jax
        import jax.numpy as jnp
        from jax.sharding import Mesh, PartitionSpec, NamedSharding
        from jax.experimental.shard_map import shard_map
        from concourse import bass2jax

        nc = _get_nc()
        bass2jax.install_neuronx_cc_hook()

        part_name = (nc.partition_id_tensor.name
                     if nc.partition_id_tensor else None)
        in_names, out_names, out_avals = [], [], []
        for alloc in nc.m.functions[0].allocations:
            if not isinstance(alloc, mybir.MemoryLocationSet):
                continue
            name = alloc.memorylocations[0].name
            if alloc.kind == "ExternalInput":
                if name != part_name:
                    in_names.append(name)
            elif alloc.kind == "ExternalOutput":
                out_names.append(name)
                out_avals.append(jax.core.ShapedArray(
                    tuple(alloc.tensor_shape), mybir.dt.np(alloc.dtype)))
        n_params = len(in_names)
        all_names = tuple(in_names) + tuple(out_names)
        if part_name is not None:
            all_names = all_names + (part_name,)
        donate = tuple(range(n_params, n_params + len(out_names)))

        def _body(*args):
            operands = list(args)
            if part_name is not None:
                operands.append(bass2jax.partition_id_tensor())
            outs = bass2jax._bass_exec_p.bind(
                *operands,
                out_avals=tuple(out_avals),
                in_names=all_names,
                out_names=tuple(out_names),
                lowering_input_output_aliases=(),
                sim_require_finite=True,
                sim_require_nnan=True,
                nc=nc,
            )
            return tuple(outs)

        devices = jax.devices()[:N_CORES]
        mesh = Mesh(np.asarray(devices), ("core",))
        spec = PartitionSpec("core")
        fn = jax.jit(
            shard_map(_body, mesh=mesh,
                      in_specs=(spec,) * (n_params + len(out_names)),
                      out_specs=(spec,) * len(out_names), check_rep=False),
            donate_argnums=donate, keep_unused=True)
        zmk = jax.jit(
            lambda: jnp.zeros((N_CORES * 512, 2048), ml_dtypes.bfloat16),
            out_shardings=NamedSharding(mesh, spec))
        _RUNNER = (fn, zmk, in_names)
    return _RUNNER


def _host_prep(x, tree_params, tree_weights):
    """Host-side: transpose/group x (fp8), pack replicated params, and fold
    the leaf-distribution combination matrices (incl. softmax) plus the
    rank-1 output shift S into precomputed arrays."""
    x = np.asarray(x, np.float32)
    # xt[(g p), (kk ko b)] = x_core[g*512 + b, kk*256 + ko*128 + p], per core
    xt = np.ascontiguousarray(
        x.reshape(N_CORES, NG, 512, 2, 2, 128).transpose(0, 1, 5, 3, 4, 2)
    ).reshape(N_CORES * 512, 2048).astype(F8NP)

    p = np.asarray(tree_params, np.float32)[0].reshape(N_TREES, PPT)
    w4 = p[:, :NW].reshape(N_TREES, N_INTERNAL, INPUT_DIM)[:, :4, :]
    wj = w4.transpose(1, 0, 2).reshape(256, INPUT_DIM)      # j = i*64 + t
    # wt8[p, kk, ko, j] -> [128, 1024]
    wt8 = np.ascontiguousarray(
        wj.T.reshape(2, 2, 128, 256).transpose(2, 0, 1, 3)).reshape(128, 1024)

    ll = p[:, NW + N_INTERNAL:].reshape(N_TREES, N_LEAVES, N_CLASSES)
    e = np.exp(ll - ll.max(axis=-1, keepdims=True))
    M = e / e.sum(axis=-1, keepdims=True)                   # softmax [T, L, C]
    M = M * np.asarray(tree_weights, np.float32)[0][:, None, None]
    C_ = M[:, 0] + M[:, 2] + M[:, 4] + M[:, 6]              # [T, C]
    G0 = M[:, 1] - M[:, 2]
    G1 = M[:, 3] - M[:, 4]
    G2 = M[:, 5] - M[:, 6]
    G3 = M[:, 7] - C_ * 0.25
    cg0 = np.concatenate([G0, G1], 0)                       # [128, C]
    cg1 = np.concatenate([G2, G3], 0)
    # cgd[p, ko, c] = 16 * cg_ko[p, c]  -> [128, 1024]
    cgd = np.stack([cg0 * 16.0, cg1 * 16.0], axis=1).reshape(128, 1024)
    pbf = np.concatenate([wt8, cgd], axis=1).astype(F8NP)   # [128, 2048]

    bias = p[:, NW:NW + N_INTERNAL][:, :4].T.reshape(256)   # j-major
    pf32 = np.zeros((128, 3), np.float32)
    pf32[:, 0] = bias[0:128]
    pf32[:, 1] = bias[128:256]
    pf32[64:128, 2] = bias[192:256] + np.float32(np.log(1.25))

    S = C_.sum(axis=0) * 0.25                               # [C] host shift
    return xt, pbf, pf32, S


def _unpermute(outd, S):
    """outd [N_CORES*512, 2048] with row g*128+p, col bt*512+c ->
    full [16384, 512] f32 plus the rank-1 shift."""
    o = outd.reshape(N_CORES, NG, 128, 4, 512).transpose(0, 1, 3, 2, 4)
    return np.ascontiguousarray(o).reshape(BATCH, N_CLASSES).astype(
        np.float32) + S[None, :]


def kernel(x: np.ndarray, tree_params: np.ndarray,
           tree_weights: np.ndarray) -> np.ndarray:
    fn, zmk, in_names = _get_runner()
    xt, pbf, pf32, S = _host_prep(x, tree_params, tree_weights)
    reps = {"xt": xt,
            "pbf": np.concatenate([pbf] * N_CORES, 0),
            "pf32": np.concatenate([pf32] * N_CORES, 0)}
    args = [reps[n] for n in in_names] + [zmk()]
    outs = fn(*args)
    return _unpermute(np.asarray(outs[0]), S)
